# revision 2
# baseline (speedup 1.0000x reference)
"""Cross-modal channel attention (CrossModelAtt) Bass/Tile kernel for TRN2.

Reference computation per batch b (C=512, HW=4096):
    q  = img[b]            # [C, N]
    kv = text[b]           # [C, N]
    S  = q @ kv.T          # [C, C]
    P  = softmax(S, -1)
    out[b] = gamma * (P @ kv) + img[b]

Sharding: pure data-parallel over batch; 16 batches / 8 cores = 2 per core.

On-core pipeline (per batch):
  - img chunks loaded fp32 (kept for the residual), cast to bf16 (gpsimd)
  - qT  built by PE tile-transposes (bf16) -> PSUM -> ACT evac
  - kv  loaded fp32, cast to bf16 (gpsimd); kvT built by DMA xbar transpose
  - mm1: S chunks [128, 512] accumulated over 32 n-chunks (bf16 matmul)
  - softmax: DVE reduce_max(negate) -> ACT exp(bias=-max, accum_out=rowsum)
             -> DVE reciprocal -> P scaled by gamma/rowsum (folds the final
             gamma multiply; mm2 then directly produces gamma*info)
  - pT by PE transposes; mm2: info chunks [128, 512] over 4 d-chunks
  - final: DVE add of img chunk (exact fp32 residual) -> DMA out
"""

import numpy as np

B, C, H, W = 16, 512, 64, 64
N = H * W                 # 4096
N_CORES = 8
BPC = B // N_CORES        # batches per core
CP = C // 128             # 4 c-chunks
NJ = N // 128             # 32 n-chunks
NB = N // 512             # 8 n-blocks

_nc_cache = None


def _build_nc():
    import concourse.mybir as mybir
    from concourse import bacc
    from concourse.tile import TileContext
    from concourse.masks import make_identity

    F32 = mybir.dt.float32
    BF16 = mybir.dt.bfloat16
    AX = mybir.AxisListType.X

    nc = bacc.Bacc("TRN2", target_bir_lowering=False, debug=False,
                   num_devices=N_CORES)

    img_d = nc.dram_tensor("img", [BPC, C, N], F32, kind="ExternalInput")
    txt_d = nc.dram_tensor("txt", [BPC, C, N], F32, kind="ExternalInput")
    gam_d = nc.dram_tensor("gam", [1], F32, kind="ExternalInput")
    out_d = nc.dram_tensor("out", [BPC, C, N], F32, kind="ExternalOutput")

    img_f = img_d.ap().flatten_outer_dims()   # [BPC*C, N]
    txt_f = txt_d.ap().flatten_outer_dims()
    out_f = out_d.ap().flatten_outer_dims()

    with TileContext(nc) as tc:
        with (
            tc.tile_pool(name="const", bufs=1) as const_pool,
            tc.tile_pool(name="img", bufs=4) as img_pool,          # [128,4096] f32
            tc.tile_pool(name="kvstage", bufs=2) as kvstage_pool,  # [128,2048] f32
            tc.tile_pool(name="kvbf", bufs=4) as kvbf_pool,        # [128,4096] bf16
            tc.tile_pool(name="kvT", bufs=1) as kvT_pool,          # [128,16384] bf16
            tc.tile_pool(name="qT", bufs=1) as qT_pool,            # [128,16384] bf16
            tc.tile_pool(name="qbf", bufs=2) as qbf_pool,          # [128,2048] bf16
            tc.tile_pool(name="p", bufs=8) as p_pool,              # [128,512] bf16
            tc.tile_pool(name="pT", bufs=4) as pT_pool,            # [128,512] bf16
            tc.tile_pool(name="ot", bufs=4) as out_pool,           # [128,512] f32
            tc.tile_pool(name="stat", bufs=16) as stat_pool,       # [128,1] f32
            tc.tile_pool(name="tps", bufs=2, space="PSUM") as tps_pool,   # transposes
            tc.tile_pool(name="sps", bufs=2, space="PSUM") as s_pool,     # S
            tc.tile_pool(name="ips", bufs=3, space="PSUM") as info_pool,  # info
        ):
            ident = const_pool.tile([128, 128], BF16, tag="ident")
            make_identity(nc, ident[:])
            # gamma broadcast to all partitions, once
            g1 = const_pool.tile([128, 1], F32, tag="g1")
            nc.sync.dma_start(g1[0:1, 0:1],
                              gam_d.ap().rearrange("(a b) -> a b", a=1))
            g_b = const_pool.tile([128, 1], F32, tag="gb")
            nc.gpsimd.partition_broadcast(g_b[:], g1[0:1, 0:1])

            for b in range(BPC):
                r0 = b * C  # row base in flattened [BPC*C, N]

                # ---------- loads ----------
                img_t = []
                for ci in range(CP):
                    t = img_pool.tile([128, N], F32, tag="img")
                    rows = slice(r0 + ci * 128, r0 + (ci + 1) * 128)
                    for h in range(2):
                        cols = slice(h * 2048, (h + 1) * 2048)
                        nc.sync.dma_start(t[:, cols], img_f[rows, cols])
                    img_t.append(t)

                kv_bf = []
                for ci in range(CP):
                    t = kvbf_pool.tile([128, N], BF16, tag="kvbf")
                    rows = slice(r0 + ci * 128, r0 + (ci + 1) * 128)
                    for h in range(4):
                        cols = slice(h * 1024, (h + 1) * 1024)
                        st = kvstage_pool.tile([128, 1024], F32, tag="kvstage")
                        nc.sync.dma_start(st[:], txt_f[rows, cols])
                        nc.gpsimd.tensor_copy(t[:, cols], st[:])
                    kv_bf.append(t)

                # ---------- kvT via DMA xbar transpose ----------
                # kvT layout: 32 tiles [128(n), 512(d)] packed in one big tile;
                # tile nj lives at free offset nj*512.
                kvT = kvT_pool.tile([128, NJ * 512], BF16, tag="kvT")
                kvTv = kvT[:].rearrange("p (e c) -> p e c", c=512)
                for ci in range(CP):
                    # out[p, e, c] = in[c, e*128 + p]
                    nc.sync.dma_start_transpose(
                        kvTv[:, :, ci * 128:(ci + 1) * 128], kv_bf[ci][:])

                # ---------- qT via PE transposes ----------
                qT = qT_pool.tile([128, NJ * 512], BF16, tag="qT")
                for ci in range(CP):
                    rows = slice(r0 + ci * 128, r0 + (ci + 1) * 128)
                    for h in range(2):
                        cols = slice(h * 2048, (h + 1) * 2048)
                        qb = qbf_pool.tile([128, 2048], BF16, tag="qbf")
                        nc.gpsimd.tensor_copy(qb[:], img_t[ci][:, cols])
                        for g in range(4):  # groups of 4 n-tiles per psum bank
                            ps = tps_pool.tile([128, 512], BF16, tag="tps")
                            for k in range(4):
                                nc.tensor.transpose(
                                    ps[:, k * 128:(k + 1) * 128],
                                    qb[:, (g * 4 + k) * 128:(g * 4 + k + 1) * 128],
                                    ident[:])
                            # scatter the 4 transposed tiles to their nj slots
                            njbase = h * 16 + g * 4
                            dst = qT[:].rearrange("p (e c) -> p e c", c=512)[
                                :, njbase:njbase + 4, ci * 128:(ci + 1) * 128]
                            src = ps[:].rearrange("p (e c) -> p e c", c=128)
                            nc.scalar.copy(dst, src)

                # ---------- mm1 + softmax ----------
                p_s = []
                for ci in range(CP):
                    s_ps = s_pool.tile([128, 512], F32, tag="s")
                    for nj in range(NJ):
                        nc.tensor.matmul(
                            s_ps[:],
                            qT[:, nj * 512 + ci * 128: nj * 512 + (ci + 1) * 128],
                            kvT[:, nj * 512:(nj + 1) * 512],
                            start=(nj == 0), stop=(nj == NJ - 1))
                    nm = stat_pool.tile([128, 1], F32, tag="nm")
                    nc.vector.reduce_max(nm[:], s_ps[:], axis=AX, negate=True)
                    pb = p_pool.tile([128, 512], BF16, tag="pb")
                    rs = stat_pool.tile([128, 1], F32, tag="rs")
                    nc.scalar.activation(pb[:], s_ps[:],
                                         mybir.ActivationFunctionType.Exp,
                                         bias=nm[:], scale=1.0, accum_out=rs[:])
                    rr = stat_pool.tile([128, 1], F32, tag="rr")
                    nc.vector.reciprocal(rr[:], rs[:])
                    rg = stat_pool.tile([128, 1], F32, tag="rg")
                    nc.vector.tensor_mul(rg[:], rr[:], g_b[:])
                    pscl = p_pool.tile([128, 512], BF16, tag="ps")
                    nc.vector.tensor_scalar_mul(pscl[:], pb[:], rg[:])
                    p_s.append(pscl)

                # ---------- pT ----------
                pT = []
                for di in range(CP):
                    ps = tps_pool.tile([128, 512], BF16, tag="tps")
                    for ci in range(CP):
                        nc.tensor.transpose(
                            ps[:, ci * 128:(ci + 1) * 128],
                            p_s[ci][:, di * 128:(di + 1) * 128],
                            ident[:])
                    t = pT_pool.tile([128, 512], BF16, tag="pT")
                    nc.scalar.copy(t[:], ps[:])
                    pT.append(t)

                # ---------- mm2 + residual + store ----------
                for ci in range(CP):
                    rows = slice(r0 + ci * 128, r0 + (ci + 1) * 128)
                    for nb in range(NB):
                        cols = slice(nb * 512, (nb + 1) * 512)
                        i_ps = info_pool.tile([128, 512], F32, tag="i")
                        for di in range(CP):
                            nc.tensor.matmul(
                                i_ps[:],
                                pT[di][:, ci * 128:(ci + 1) * 128],
                                kv_bf[di][:, cols],
                                start=(di == 0), stop=(di == CP - 1))
                        ot = out_pool.tile([128, 512], F32, tag="ot")
                        nc.vector.tensor_add(ot[:], i_ps[:], img_t[ci][:, cols])
                        nc.sync.dma_start(out_f[rows, cols], ot[:])

    nc.compile()
    return nc


def _get_nc():
    global _nc_cache
    if _nc_cache is None:
        _nc_cache = _build_nc()
    return _nc_cache


def kernel(img_feat, text_feat, gamma):
    from concourse.bass_utils import run_bass_kernel_spmd

    nc = _get_nc()
    img = np.ascontiguousarray(np.asarray(img_feat), dtype=np.float32).reshape(B, C, N)
    txt = np.ascontiguousarray(np.asarray(text_feat), dtype=np.float32).reshape(B, C, N)
    g = np.ascontiguousarray(np.asarray(gamma), dtype=np.float32).reshape(1)

    in_maps = [
        {
            "img": img[i * BPC:(i + 1) * BPC],
            "txt": txt[i * BPC:(i + 1) * BPC],
            "gam": g,
        }
        for i in range(N_CORES)
    ]
    res = run_bass_kernel_spmd(nc, in_maps, core_ids=list(range(N_CORES)))
    out = np.concatenate([res.results[i]["out"] for i in range(N_CORES)], axis=0)
    return out.reshape(B, C, H, W).astype(np.float32)
